# revision 1
# baseline (speedup 1.0000x reference)
"""Trainium2 Bass kernel for nn_Custom_trainer_79242146611896.

Data-parallel over the N=16384 sample dim across 8 NeuronCores
(2048 samples/core); per-class segment sums ([C,D] + counts + 3 scalar
partials) are combined with a single on-device AllReduce; the small
weight matrices are replicated.

Per-core computation (all matmuls on TensorE in f32r; activations in
f32 on ScalarE/VectorE):
  encodedT = tanh(W_enc^T x^T)       via PE-transposed x tiles
  decodedT = W_dec^T encodedT        (+ b_dec)
  rec_latents = tanh(decoded W_enc)  (+ b_enc via K=1 matmul)
  logits/softmax/CCE, pinball sums (= 0.9*|diff| sums), segment sums
  via onehot matmuls, then after the AllReduce: per-class means and the
  per-sample within-group sum of squares.
"""

import numpy as np

import concourse.bass as bass
import concourse.mybir as mybir
import concourse.tile as tile
from concourse import bacc
from concourse.bass_utils import run_bass_kernel_spmd
from concourse.masks import make_identity

F32 = mybir.dt.float32
F32R = mybir.dt.float32r
BF16 = mybir.dt.bfloat16
I32 = mybir.dt.int32
AX = mybir.AxisListType
ALU = mybir.AluOpType
ACTF = mybir.ActivationFunctionType

P = 128
NCORES = 8
N_GLOBAL = 16384
T = 2048
D = 512
C = 50
KEPS = 1e-7

MM_DT = F32R  # matmul operand dtype: F32R | BF16 | F32


def build(nl=N_GLOBAL // NCORES, nc_chunk=256, mm_dt=MM_DT, n_global=None):
    n_global = n_global or NCORES * nl
    NT = T // P          # 16 T-tiles
    ND = D // P          # 4 D-tiles
    NN = nl // P         # n-tiles per core
    NC = nc_chunk        # samples per chunk
    NCH = nl // NC       # chunks
    NSUB = NC // P       # n-tiles per chunk

    nc = bacc.Bacc("TRN2", target_bir_lowering=False, debug=False, num_devices=NCORES)

    x_d = nc.dram_tensor("x", [nl, T], F32, kind="ExternalInput")
    o_d = nc.dram_tensor("output", [nl, T], F32, kind="ExternalInput")
    cl_d = nc.dram_tensor("cat_labels", [nl, C], F32, kind="ExternalInput")
    lab_d = nc.dram_tensor("labels", [nl], I32, kind="ExternalInput")
    wenc_d = nc.dram_tensor("W_enc", [T, D], F32, kind="ExternalInput")
    benc_d = nc.dram_tensor("b_enc", [D], F32, kind="ExternalInput")
    wdec_d = nc.dram_tensor("W_dec", [D, T], F32, kind="ExternalInput")
    bdec_d = nc.dram_tensor("b_dec", [T], F32, kind="ExternalInput")
    wcls_d = nc.dram_tensor("W_cls", [D, C], F32, kind="ExternalInput")
    bcls_d = nc.dram_tensor("b_cls", [C], F32, kind="ExternalInput")
    out_d = nc.dram_tensor("out", [nl], F32, kind="ExternalOutput")

    cast_weights = mm_dt != F32

    from contextlib import ExitStack

    with tile.TileContext(nc) as tc:
        with ExitStack() as ctx:
            ent = ctx.enter_context
            constp = ent(tc.tile_pool(name="const", bufs=1))   # identities, ones, iota
            wts = ent(tc.tile_pool(name="wts", bufs=1))        # persistent weights
            encp = ent(tc.tile_pool(name="enc", bufs=1))       # persistent encodedT
            accp = ent(tc.tile_pool(name="acc", bufs=1))       # strips + accumulators
            stg = ent(tc.tile_pool(name="stg", bufs=1))        # weight-cast staging
            xrowp = ent(tc.tile_pool(name="xrow", bufs=2))
            orowp = ent(tc.tile_pool(name="orow", bufs=2))
            xtp = ent(tc.tile_pool(name="xt", bufs=NT))
            dctp = ent(tc.tile_pool(name="dct", bufs=NT))
            ennatp = ent(tc.tile_pool(name="ennat", bufs=3))
            latp = ent(tc.tile_pool(name="lat", bufs=2))
            big5p = ent(tc.tile_pool(name="big5", bufs=2))     # [128,512] scratch
            dsubp = ent(tc.tile_pool(name="dsub", bufs=3))
            junkdp = ent(tc.tile_pool(name="junkd", bufs=2))
            smallp = ent(tc.tile_pool(name="small", bufs=6))   # [128,50]-ish scratch
            colsp = ent(tc.tile_pool(name="cols", bufs=2))    # [128,1] scratch
            catlp = ent(tc.tile_pool(name="catl", bufs=3))
            psm = ent(tc.tile_pool(name="psm", bufs=3, space="PSUM"))   # [128,512] matmul psum
            pst = ent(tc.tile_pool(name="pst", bufs=3, space="PSUM"))   # [128,128] transpose psum
            dp = ent(tc.tile_pool(name="dram", bufs=1, space="DRAM"))
            # ---------------- constants & weights ----------------
            ident_f32 = constp.tile([P, P], F32)
            make_identity(nc, ident_f32)
            if cast_weights:
                ident_mm = constp.tile([P, P], mm_dt)
                nc.vector.tensor_copy(ident_mm[:], ident_f32[:])
            else:
                ident_mm = ident_f32

            ones_col = constp.tile([P, 1], F32)
            nc.any.memset(ones_col[:], 1.0)
            ones_k1f = constp.tile([1, P], F32)
            nc.any.memset(ones_k1f[:], 1.0)
            if cast_weights:
                ones_k1 = constp.tile([1, P], mm_dt)
                nc.vector.tensor_copy(ones_k1[:], ones_k1f[:])
            else:
                ones_k1 = ones_k1f

            iot = constp.tile([P, C], I32)
            nc.gpsimd.iota(iot[:], [[1, C]], channel_multiplier=0)
            iotaf = constp.tile([P, C], F32)
            nc.vector.tensor_copy(iotaf[:], iot[:])

            def load_cast(pool, shape, dram_ap, tag, dt_=None):
                dt_ = dt_ or mm_dt
                if not cast_weights or dt_ == F32:
                    t_ = pool.tile(shape, F32, name=tag, tag=tag)
                    nc.sync.dma_start(t_[:], dram_ap)
                    return t_
                s_ = stg.tile(list(shape), F32, name="stg", tag="stg")
                nc.sync.dma_start(s_[:], dram_ap)
                t_ = pool.tile(shape, dt_, name=tag, tag=tag)
                nc.vector.tensor_copy(t_[:], s_[:])
                return t_

            wenc_r = wenc_d.ap().rearrange("(a p) d -> a p d", p=P)
            wenc = [load_cast(wts, [P, D], wenc_r[t], f"wenc{t}") for t in range(NT)]
            wdec_r = wdec_d.ap().rearrange("(a p) t -> a p t", p=P)
            wdec = [load_cast(wts, [P, T], wdec_r[k], f"wdec{k}") for k in range(ND)]
            wcls_r = wcls_d.ap().rearrange("(a p) c -> a p c", p=P)
            wcls = [load_cast(wts, [P, C], wcls_r[k], f"wcls{k}") for k in range(ND)]

            benc_r = benc_d.ap().rearrange("(a p) -> a p", p=P)
            bencT = []
            for k in range(ND):
                b_ = wts.tile([P, 1], F32, tag=f"bencT{k}")
                nc.sync.dma_start(b_[:], benc_r[k].rearrange("(p o) -> p o", o=1))
                bencT.append(b_)
            benc_row = load_cast(wts, [1, D], benc_d.ap().rearrange("(o d) -> o d", o=1), "benc_row")
            bdec_r = bdec_d.ap().rearrange("(a p) -> a p", p=P)
            bdecT = []
            for t in range(NT):
                b_ = wts.tile([P, 1], F32, tag=f"bdecT{t}")
                nc.sync.dma_start(b_[:], bdec_r[t].rearrange("(p o) -> p o", o=1))
                bdecT.append(b_)
            bcls_row = load_cast(wts, [1, C], bcls_d.ap().rearrange("(o c) -> o c", o=1), "bcls_row")

            encT = [encp.tile([P, nl], mm_dt, name=f"encT{k}", tag=f"encT{k}") for k in range(ND)]

            rec_strip = accp.tile([P, NCH * NT * NSUB], F32)
            lat_strip = accp.tile([P, NN], F32)
            cat_strip = accp.tile([P, NN], F32)
            nsq_strip = accp.tile([P, NN], F32)
            seg_sb = accp.tile([C, D], F32)
            labfs = [accp.tile([P, 1], F32, name=f"labf{i}", tag=f"labf{i}") for i in range(NN)]
            onehot = [accp.tile([P, C], F32, name=f"oh{i}", tag=f"oh{i}") for i in range(NN)]

            enc_nat = {}

            # ================= phase 1: chunk loop =================
            for c in range(NCH):
                base = c * NC

                xr = []
                for s in range(NSUB):
                    r_ = xrowp.tile([P, T], F32, tag="xrow")
                    nc.sync.dma_start(r_[:], x_d[base + s * P : base + (s + 1) * P, :])
                    xr.append(r_)

                # transpose x -> xT tiles [128T, NC]
                xt = []
                for t in range(NT):
                    xt_t = xtp.tile([P, NC], mm_dt, tag="xt")
                    for s in range(NSUB):
                        tp = pst.tile([P, P], F32, tag="pst")
                        nc.tensor.transpose(
                            tp[:], xr[s][:, t * P : (t + 1) * P], ident_f32[:]
                        )
                        if (t + s) % 2 == 0:
                            nc.scalar.activation(
                                xt_t[:, s * P : (s + 1) * P], tp[:], ACTF.Copy
                            )
                        else:
                            nc.vector.tensor_copy(
                                xt_t[:, s * P : (s + 1) * P], tp[:]
                            )
                    xt.append(xt_t)

                # mm1: encodedT[:, chunk] = tanh(W_enc^T xT + b_enc)
                for k in range(ND):
                    ps = psm.tile([P, NC], F32, tag="psm")
                    for t in range(NT):
                        nc.tensor.matmul(
                            ps[:], wenc[t][:, k * P : (k + 1) * P], xt[t][:],
                            start=(t == 0), stop=(t == NT - 1),
                        )
                    nc.scalar.activation(
                        encT[k][:, base : base + NC], ps[:], ACTF.Tanh, bias=bencT[k][:]
                    )

                # enc natural tiles + labels/onehot + segment sums + normsq
                for s in range(NSUB):
                    i = c * NSUB + s
                    en = ennatp.tile([P, D], F32, tag="ennat")
                    for k in range(ND):
                        tp = pst.tile([P, P], mm_dt, tag="pst")
                        nc.tensor.transpose(
                            tp[:], encT[k][:, base + s * P : base + (s + 1) * P],
                            ident_mm[:],
                        )
                        nc.vector.tensor_copy(en[:, k * P : (k + 1) * P], tp[:])
                    enc_nat[i] = en

                    labi = colsp.tile([P, 1], I32, tag="labi")
                    nc.sync.dma_start(
                        labi[:], lab_d[i * P : (i + 1) * P].rearrange("(p o) -> p o", o=1)
                    )
                    nc.vector.tensor_copy(labfs[i][:], labi[:])
                    nc.vector.tensor_scalar(
                        out=onehot[i][:], in0=iotaf[:], scalar1=labfs[i][:],
                        scalar2=None, op0=ALU.is_equal,
                    )

                    sps = psm.tile([C, D], F32, tag="psm")
                    nc.tensor.matmul(sps[:], onehot[i][:], en[:], start=True, stop=True)
                    if i == 0:
                        nc.vector.tensor_copy(seg_sb[:], sps[:])
                    else:
                        nc.vector.tensor_tensor(seg_sb[:], seg_sb[:], sps[:], ALU.add)

                    jn = big5p.tile([P, D], F32, tag="big5")
                    nc.scalar.activation(
                        jn[:], en[:], ACTF.Square, accum_out=nsq_strip[:, i : i + 1]
                    )

                # mm2: decodedT tiles [128T, NC]
                dct = []
                for t in range(NT):
                    ps = psm.tile([P, NC], F32, tag="psm")
                    for k in range(ND):
                        nc.tensor.matmul(
                            ps[:], wdec[k][:, t * P : (t + 1) * P],
                            encT[k][:, base : base + NC],
                            start=(k == 0), stop=(k == ND - 1),
                        )
                    d_t = dctp.tile([P, NC], mm_dt, tag="dct")
                    if t % 2 == 0:
                        nc.scalar.activation(
                            d_t[:], ps[:], ACTF.Identity, bias=bdecT[t][:]
                        )
                    else:
                        nc.vector.tensor_scalar(
                            out=d_t[:], in0=ps[:], scalar1=bdecT[t][:],
                            scalar2=None, op0=ALU.add,
                        )
                    dct.append(d_t)

                # rec pinball: |decoded - output| summed
                orow = []
                for s in range(NSUB):
                    r_ = orowp.tile([P, T], F32, tag="orow")
                    nc.sync.dma_start(r_[:], o_d[base + s * P : base + (s + 1) * P, :])
                    orow.append(r_)
                for t in range(NT):
                    for s in range(NSUB):
                        tp = pst.tile([P, P], mm_dt, tag="pst")
                        nc.tensor.transpose(
                            tp[:], dct[t][:, s * P : (s + 1) * P], ident_mm[:]
                        )
                        dd = dsubp.tile([P, P], F32, tag="dsub")
                        nc.vector.tensor_tensor(
                            dd[:], tp[:], orow[s][:, t * P : (t + 1) * P], ALU.subtract
                        )
                        col = c * NT * NSUB + t * NSUB + s
                        nc.vector.tensor_reduce(
                            rec_strip[:, col : col + 1], dd[:], AX.X, ALU.add,
                            apply_absolute_value=True,
                        )

                # mm4: rec_latents = tanh(decoded @ W_enc + b_enc); lat pinball
                for s in range(NSUB):
                    i = c * NSUB + s
                    ps = psm.tile([P, D], F32, tag="psm")
                    for t in range(NT):
                        nc.tensor.matmul(
                            ps[:], dct[t][:, s * P : (s + 1) * P], wenc[t][:],
                            start=(t == 0), stop=False,
                        )
                    nc.tensor.matmul(
                        ps[:], ones_k1[:], benc_row[:], start=False, stop=True
                    )
                    lt = latp.tile([P, D], F32, tag="lat")
                    nc.scalar.activation(lt[:], ps[:], ACTF.Tanh)
                    d2 = big5p.tile([P, D], F32, tag="big5")
                    nc.vector.tensor_tensor(d2[:], lt[:], enc_nat[i][:], ALU.subtract)
                    nc.vector.tensor_reduce(
                        lat_strip[:, i : i + 1], d2[:], AX.X, ALU.add,
                        apply_absolute_value=True,
                    )

                # mm3: logits -> softmax -> swapped-arg CCE
                for s in range(NSUB):
                    i = c * NSUB + s
                    ps = psm.tile([P, C], F32, tag="psm")
                    for k in range(ND):
                        nc.tensor.matmul(
                            ps[:], encT[k][:, base + s * P : base + (s + 1) * P],
                            wcls[k][:], start=(k == 0), stop=False,
                        )
                    nc.tensor.matmul(
                        ps[:], ones_k1[:], bcls_row[:], start=False, stop=True
                    )
                    nmx = colsp.tile([P, 1], F32, tag="nmx")
                    nc.vector.tensor_reduce(nmx[:], ps[:], AX.X, ALU.max, negate=True)
                    expt = smallp.tile([P, C], F32, tag="small")
                    sume = colsp.tile([P, 1], F32, tag="sume")
                    nc.scalar.activation(
                        expt[:], ps[:], ACTF.Exp, bias=nmx[:], accum_out=sume[:]
                    )
                    rcp = colsp.tile([P, 1], F32, tag="rcp")
                    nc.vector.reciprocal(rcp[:], sume[:])

                    cl = catlp.tile([P, C], F32, tag="catl")
                    nc.sync.dma_start(cl[:], cl_d[i * P : (i + 1) * P, :])
                    rs = colsp.tile([P, 1], F32, tag="rs")
                    nc.vector.tensor_reduce(rs[:], cl[:], AX.X, ALU.add)
                    rr = colsp.tile([P, 1], F32, tag="rr")
                    nc.vector.reciprocal(rr[:], rs[:])
                    yp = smallp.tile([P, C], F32, tag="small")
                    nc.vector.tensor_scalar(
                        out=yp[:], in0=cl[:], scalar1=rr[:], scalar2=None, op0=ALU.mult
                    )
                    ypc = smallp.tile([P, C], F32, tag="small")
                    nc.vector.tensor_scalar(
                        out=ypc[:], in0=yp[:], scalar1=KEPS, scalar2=1.0 - KEPS,
                        op0=ALU.max, op1=ALU.min,
                    )
                    lg = smallp.tile([P, C], F32, tag="small")
                    nc.scalar.activation(lg[:], ypc[:], ACTF.Ln)
                    t1 = colsp.tile([P, 1], F32, tag="t1")
                    j3 = smallp.tile([P, C], F32, tag="small")
                    nc.vector.scalar_tensor_tensor(
                        out=j3[:], in0=expt[:], scalar=0.0, in1=lg[:],
                        op0=ALU.bypass, op1=ALU.mult, accum_out=t1[:],
                    )
                    nc.vector.tensor_scalar(
                        out=cat_strip[:, i : i + 1], in0=t1[:], scalar1=rcp[:],
                        scalar2=-1.0, op0=ALU.mult, op1=ALU.mult,
                    )

            # ================= phase 1 tail =================
            cps = psm.tile([C, 1], F32, tag="psm")
            for i in range(NN):
                nc.tensor.matmul(
                    cps[:], onehot[i][:], ones_col[:],
                    start=(i == 0), stop=(i == NN - 1),
                )
            counts_sb = accp.tile([C, 1], F32)
            nc.scalar.activation(counts_sb[:], cps[:], ACTF.Copy)

            pack3 = accp.tile([P, 3], F32)
            nc.vector.tensor_reduce(pack3[:, 0:1], rec_strip[:], AX.X, ALU.add)
            nc.vector.tensor_reduce(pack3[:, 1:2], lat_strip[:], AX.X, ALU.add)
            nc.vector.tensor_reduce(pack3[:, 2:3], cat_strip[:], AX.X, ALU.add)
            scps = psm.tile([1, 3], F32, tag="psm")
            nc.tensor.matmul(scps[:], ones_col[:], pack3[:], start=True, stop=True)
            sc_row = accp.tile([1, 3], F32)
            nc.scalar.activation(sc_row[:], scps[:], ACTF.Copy)

            bounce_in = dp.tile([C, 516], F32)
            bounce_out = dp.tile([C, 516], F32)
            zr4 = accp.tile([C, 4], F32)
            nc.any.memset(zr4[:], 0.0)
            nc.sync.dma_start(bounce_in[:, D : D + 4], zr4[:])
            nc.sync.dma_start(bounce_in[:, 0:D], seg_sb[:])
            nc.sync.dma_start(bounce_in[:, D : D + 1], counts_sb[:])
            nc.sync.dma_start(bounce_in[0:1, D + 1 : D + 4], sc_row[:])
            nc.gpsimd.collective_compute(
                "AllReduce",
                ALU.add,
                replica_groups=[list(range(NCORES))],
                ins=[bounce_in[:].opt()],
                outs=[bounce_out[:].opt()],
            )
            sums_g = accp.tile([C, D], F32)
            nc.sync.dma_start(sums_g[:], bounce_out[:, 0:D])
            counts_g = accp.tile([C, 1], F32)
            nc.sync.dma_start(counts_g[:], bounce_out[:, D : D + 1])
            sc_g = accp.tile([1, 3], F32)
            nc.sync.dma_start(sc_g[:], bounce_out[0:1, D + 1 : D + 4])

            # ================= phase 2 =================
            cmax = accp.tile([C, 1], F32)
            nc.vector.tensor_scalar(
                out=cmax[:], in0=counts_g[:], scalar1=1.0, scalar2=None, op0=ALU.max
            )
            crcp = accp.tile([C, 1], F32)
            nc.vector.reciprocal(crcp[:], cmax[:])
            means = accp.tile([C, D], F32)
            nc.vector.tensor_scalar(
                out=means[:], in0=sums_g[:], scalar1=crcp[:], scalar2=None, op0=ALU.mult
            )
            msq_col = accp.tile([C, 1], F32)
            jm = big5p.tile([C, D], F32, tag="big5")
            nc.scalar.activation(jm[:], means[:], ACTF.Square, accum_out=msq_col[:])

            meansT = []
            for k in range(ND):
                tp = pst.tile([P, C], F32, tag="pst")
                nc.tensor.transpose(
                    tp[:], means[:, k * P : (k + 1) * P], ident_f32[:C, :C]
                )
                mt = accp.tile([P, C], mm_dt, tag=f"meansT{k}")
                nc.scalar.activation(mt[:], tp[:], ACTF.Copy)
                meansT.append(mt)

            tpm = pst.tile([1, C], F32, tag="pst")
            nc.tensor.transpose(tpm[:], msq_col[:], ident_f32[:C, :C])
            msq_row = accp.tile([1, C], F32)
            nc.scalar.activation(msq_row[:], tpm[:], ACTF.Copy)
            psb = psm.tile([P, C], F32, tag="psm")
            nc.tensor.matmul(psb[:], ones_k1f[:], msq_row[:], start=True, stop=True)
            msq_b = accp.tile([P, C], F32)
            nc.scalar.activation(msq_b[:], psb[:], ACTF.Copy)

            coef = accp.tile([1, 3], F32)
            nc.any.memset(coef[:, 0:1], 0.9 / (n_global * T))
            nc.any.memset(coef[:, 1:2], 0.9 / (n_global * D))
            nc.any.memset(coef[:, 2:3], 1.0 / n_global)
            sprod = accp.tile([1, 3], F32)
            nc.vector.tensor_tensor(sprod[:], sc_g[:], coef[:], ALU.mult)
            stot = accp.tile([1, 1], F32)
            nc.vector.tensor_reduce(stot[:], sprod[:], AX.X, ALU.add)
            psS = psm.tile([P, 1], F32, tag="psm")
            nc.tensor.matmul(psS[:], ones_k1f[:], stot[:], start=True, stop=True)
            s_col = accp.tile([P, 1], F32)
            nc.scalar.activation(s_col[:], psS[:], ACTF.Copy)

            for i in range(NN):
                eps_ = psm.tile([P, C], F32, tag="psm")
                for k in range(ND):
                    nc.tensor.matmul(
                        eps_[:], encT[k][:, i * P : (i + 1) * P], meansT[k][:],
                        start=(k == 0), stop=(k == ND - 1),
                    )
                q = smallp.tile([P, C], F32, tag="small")
                nc.vector.scalar_tensor_tensor(
                    out=q[:], in0=eps_[:], scalar=-2.0, in1=msq_b[:],
                    op0=ALU.mult, op1=ALU.add,
                )
                gq = colsp.tile([P, 1], F32, tag="gq")
                j4 = smallp.tile([P, C], F32, tag="small")
                nc.vector.scalar_tensor_tensor(
                    out=j4[:], in0=q[:], scalar=0.0, in1=onehot[i][:],
                    op0=ALU.bypass, op1=ALU.mult, accum_out=gq[:],
                )
                t2 = colsp.tile([P, 1], F32, tag="t2")
                nc.vector.tensor_tensor(t2[:], gq[:], nsq_strip[:, i : i + 1], ALU.add)
                oc = colsp.tile([P, 1], F32, tag="oc")
                nc.vector.scalar_tensor_tensor(
                    out=oc[:], in0=t2[:], scalar=1.0 / D, in1=s_col[:],
                    op0=ALU.mult, op1=ALU.add,
                )
                nc.sync.dma_start(
                    out_d[i * P : (i + 1) * P].rearrange("(p o) -> p o", o=1), oc[:]
                )

    nc.compile()
    return nc


_CACHE = {}


def _get_nc():
    if "nc" not in _CACHE:
        _CACHE["nc"] = build()
    return _CACHE["nc"]


def kernel(**inputs):
    nc = _get_nc()
    nl = N_GLOBAL // NCORES
    shard_names = ["x", "output", "cat_labels", "labels"]
    full_names = ["W_enc", "b_enc", "W_dec", "b_dec", "W_cls", "b_cls"]
    in_maps = []
    for i in range(NCORES):
        m = {}
        for k in shard_names:
            m[k] = np.ascontiguousarray(inputs[k][i * nl : (i + 1) * nl])
        for k in full_names:
            m[k] = np.ascontiguousarray(inputs[k])
        in_maps.append(m)
    res = run_bass_kernel_spmd(nc, in_maps, list(range(NCORES))).results
    return np.concatenate([res[i]["out"] for i in range(NCORES)]).astype(np.float32)



# revision 12
# speedup vs baseline: 1.4053x; 1.4053x over previous
"""Trainium2 Bass kernel for nn_Custom_trainer_79242146611896 (v2).

Data-parallel over N=16384 samples across 8 NeuronCores (2048/core).

v2 structure (vs v1 baseline):
  - all matmuls/transposes in bf16 (1 cyc/row on PE instead of f32r's
    LOW_HIGH dual pass); x transposed in f32r mode (1.5 cyc/row, exact)
  - phase 1 split: pass A (mm1 -> encT/en + segment sums), then the
    [C,D+..] AllReduce is kicked EARLY and hides under the mm3/CCE block
    and pass B (mm2 -> decodedT, rec pinball, mm4 -> rec_latents)
  - a second tiny AllReduce for the 3 global scalar partials hides under
    the phase-2 per-class matmul loop
  - transposes grouped 4x into [128,512] PSUM tiles -> wide single-instr
    copies/subtracts/reduces on DVE/Pool/ScalarE (Pool engine was idle
    in v1)
  - segment sums accumulate in a dedicated PSUM bank across the whole
    pass (no DVE adds); counts ride a parallel chained matmul
  - all Ln ops grouped in one block: 2 activation-table swaps total
    instead of 32
  - batched phase 2 (4 sample-tiles per PSUM bank) + single output DMA
"""

import numpy as np

import concourse.bass as bass
import concourse.mybir as mybir
import concourse.tile as tile
from concourse import bacc
from concourse.bass_utils import run_bass_kernel_spmd
from concourse.masks import make_identity

F32 = mybir.dt.float32
F32R = mybir.dt.float32r
BF16 = mybir.dt.bfloat16
I32 = mybir.dt.int32
AX = mybir.AxisListType
ALU = mybir.AluOpType
ACTF = mybir.ActivationFunctionType

P = 128
NCORES = 8
N_GLOBAL = 16384
T = 2048
D = 512
C = 50
KEPS = 1e-7


def build(nl=N_GLOBAL // NCORES, nc_chunk=256, n_global=None):
    n_global = n_global or NCORES * nl
    NT = T // P          # 16 T-tiles
    ND = D // P          # 4 D-tiles
    NN = nl // P         # 16 sample-tiles per core
    NC = nc_chunk        # samples per chunk (256)
    NCH = nl // NC       # 8 chunks
    NSUB = NC // P       # 2 sample-tiles per chunk
    RSQD = 1.0 / float(np.sqrt(D))   # scale so nsq/msq come out pre-divided by D

    nc = bacc.Bacc("TRN2", target_bir_lowering=False, debug=False, num_devices=NCORES)

    # x declared f32r so PE transposes run in f32r mode (1.5 cyc/row, exact)
    x_d = nc.dram_tensor("x", [nl, T], F32R, kind="ExternalInput")
    o_d = nc.dram_tensor("output", [nl, T], F32, kind="ExternalInput")
    cl_d = nc.dram_tensor("cat_labels", [nl, C], F32, kind="ExternalInput")
    lab_d = nc.dram_tensor("labels", [nl], I32, kind="ExternalInput")
    wenc_d = nc.dram_tensor("W_enc", [T, D], F32, kind="ExternalInput")
    benc_d = nc.dram_tensor("b_enc", [D], F32, kind="ExternalInput")
    wdec_d = nc.dram_tensor("W_dec", [D, T], F32, kind="ExternalInput")
    bdec_d = nc.dram_tensor("b_dec", [T], F32, kind="ExternalInput")
    wcls_d = nc.dram_tensor("W_cls", [D, C], F32, kind="ExternalInput")
    bcls_d = nc.dram_tensor("b_cls", [C], F32, kind="ExternalInput")
    out_d = nc.dram_tensor("out", [nl], F32, kind="ExternalOutput")

    from contextlib import ExitStack

    with tile.TileContext(nc) as tc:
        with ExitStack() as ctx:
            ent = ctx.enter_context
            constp = ent(tc.tile_pool(name="const", bufs=1))
            wts = ent(tc.tile_pool(name="wts", bufs=1))      # persistent weights
            encp = ent(tc.tile_pool(name="enc", bufs=1))     # encT + en (persistent)
            accp = ent(tc.tile_pool(name="acc", bufs=1))     # strips, wides, onehot
            junkp = ent(tc.tile_pool(name="junk", bufs=2))
            orowp = ent(tc.tile_pool(name="orow", bufs=2))   # prefetch across A/B
            dp = ent(tc.tile_pool(name="dram", bufs=1, space="DRAM"))

            # ---------------- constants ----------------
            ident_f32 = constp.tile([P, P], F32)
            make_identity(nc, ident_f32)
            ident_bf = constp.tile([P, P], BF16)
            nc.vector.tensor_copy(ident_bf[:], ident_f32[:])
            ident_fr = constp.tile([P, P], F32R)
            nc.vector.tensor_copy(ident_fr[:], ident_f32[:])

            ones_col = constp.tile([P, 1], F32)
            nc.any.memset(ones_col[:], 1.0)
            ones_col_bf = constp.tile([P, 1], BF16)
            nc.any.memset(ones_col_bf[:], 1.0)
            ones_k1f = constp.tile([1, P], F32)
            nc.any.memset(ones_k1f[:], 1.0)
            ones_k1b = constp.tile([1, P], BF16)
            nc.any.memset(ones_k1b[:], 1.0)

            iot = constp.tile([P, C], I32)
            nc.gpsimd.iota(iot[:], [[1, C]], channel_multiplier=0)
            iotaf = constp.tile([P, C], F32)
            nc.vector.tensor_copy(iotaf[:], iot[:])

            # ---------------- early DMAs: labels, catl ----------------
            labi_all = accp.tile([P, NN], I32)
            for i in range(NN):
                nc.sync.dma_start(
                    labi_all[:, i : i + 1],
                    lab_d[i * P : (i + 1) * P].rearrange("(p o) -> p o", o=1),
                )
            labf_all = accp.tile([P, NN], F32)
            nc.vector.tensor_copy(labf_all[:], labi_all[:])
            oh_all = accp.tile([P, NN * C], F32)     # one-hot, f32 (phase 2)
            oh_bf = accp.tile([P, NN * C], BF16)     # one-hot, bf16 (seg matmul)
            for i in range(NN):
                nc.vector.tensor_scalar(
                    out=oh_all[:, i * C : (i + 1) * C], in0=iotaf[:],
                    scalar1=labf_all[:, i : i + 1], scalar2=None, op0=ALU.is_equal,
                )
            nc.gpsimd.tensor_copy(oh_bf[:], oh_all[:])

            # catl for the whole core: [128, NN*C] (n-major blocks)
            catl_all = accp.tile([P, NN * C], F32)
            nc.sync.dma_start(
                catl_all[:],
                cl_d.ap().rearrange("(a p) c -> p a c", p=P),
            )

            # persistent big tensors
            encT = [encp.tile([P, nl], BF16, name=f"encT{k}", tag=f"encT{k}")
                    for k in range(ND)]
            en_t = [encp.tile([P, D], BF16, name=f"en{i}", tag=f"en{i}")
                    for i in range(NN)]

            nsq_strip = accp.tile([P, NN], F32)      # sum(enc^2)/D per sample
            rec_strip = accp.tile([P, NCH * 8], F32)
            lat_strip = accp.tile([P, NN], F32)
            cat_strip = accp.tile([P, NN], F32)

            # ======== weights (scoped staging) + PASS A ========
            with tc.tile_pool(name="stg", bufs=2) as stg, \
                 tc.tile_pool(name="xr", bufs=2) as xrp, \
                 tc.tile_pool(name="xt", bufs=2) as xtp, \
                 tc.tile_pool(name="ps_xt", bufs=2, space="PSUM") as ps_xt, \
                 tc.tile_pool(name="ps_mm1", bufs=2, space="PSUM") as ps_mm1, \
                 tc.tile_pool(name="ps_en", bufs=1, space="PSUM") as ps_en, \
                 tc.tile_pool(name="ps_seg", bufs=1, space="PSUM") as ps_seg, \
                 tc.tile_pool(name="ps_misc", bufs=1, space="PSUM") as ps_miscp:

                def load_x(c):
                    rs = []
                    for s in range(NSUB):
                        r_ = xrp.tile([P, T], F32R, tag=f"xr{s}")
                        nc.sync.dma_start(
                            r_[:], x_d[c * NC + s * P : c * NC + (s + 1) * P, :]
                        )
                        rs.append(r_)
                    return rs

                xrow_c = load_x(0)

                # ---- weights: f32 load + bf16 cast ----
                wenc_r = wenc_d.ap().rearrange("(a p) d -> a p d", p=P)
                wenc = []
                for t in range(NT):
                    s_ = stg.tile([P, D], F32, tag="stgd")
                    nc.sync.dma_start(s_[:], wenc_r[t])
                    w_ = wts.tile([P, D], BF16, tag=f"wenc{t}")
                    (nc.gpsimd if t % 2 else nc.vector).tensor_copy(w_[:], s_[:])
                    wenc.append(w_)

                benc_r = benc_d.ap().rearrange("(a p) -> a p", p=P)
                bencT = []
                for k in range(ND):
                    b_ = wts.tile([P, 1], F32, tag=f"bencT{k}")
                    nc.sync.dma_start(b_[:], benc_r[k].rearrange("(p o) -> p o", o=1))
                    bencT.append(b_)
                benc_row_f = wts.tile([1, D], F32)
                nc.sync.dma_start(
                    benc_row_f[:], benc_d.ap().rearrange("(o d) -> o d", o=1)
                )
                benc_row = wts.tile([1, D], BF16)
                nc.vector.tensor_copy(benc_row[:], benc_row_f[:])

                wdec_r = wdec_d.ap().rearrange("(a p) t -> a p t", p=P)
                wdec = []
                for k in range(ND):
                    s_ = stg.tile([P, T], F32, tag="stgt")
                    nc.sync.dma_start(s_[:], wdec_r[k])
                    w_ = wts.tile([P, T], BF16, tag=f"wdec{k}")
                    (nc.gpsimd if k % 2 else nc.vector).tensor_copy(w_[:], s_[:])
                    wdec.append(w_)

                bdec_r = bdec_d.ap().rearrange("(a p) -> a p", p=P)
                bdecT = []
                for t in range(NT):
                    b_ = wts.tile([P, 1], F32, tag=f"bdecT{t}")
                    nc.sync.dma_start(b_[:], bdec_r[t].rearrange("(p o) -> p o", o=1))
                    bdecT.append(b_)

                wcls_r = wcls_d.ap().rearrange("(a p) c -> a p c", p=P)
                wcls = []
                for k in range(ND):
                    s_ = stg.tile([P, C], F32, tag="stgc")
                    nc.sync.dma_start(s_[:], wcls_r[k])
                    w_ = wts.tile([P, C], BF16, tag=f"wcls{k}")
                    nc.vector.tensor_copy(w_[:], s_[:])
                    wcls.append(w_)
                bcls_row_f = wts.tile([1, C], F32)
                nc.sync.dma_start(
                    bcls_row_f[:], bcls_d.ap().rearrange("(o c) -> o c", o=1)
                )
                bcls_row = wts.tile([1, C], BF16)
                nc.vector.tensor_copy(bcls_row[:], bcls_row_f[:])

                # ---- PASS A chunks: x^T, mm1 -> encT, en, nsq ----
                for c in range(NCH):
                    base = c * NC
                    xr = xrow_c
                    if c + 1 < NCH:
                        xrow_nxt = load_x(c + 1)

                    # x transposes (f32r, 1.5 cyc/row) grouped 4x into [128,512]
                    xt_all = xtp.tile([P, NT * NC], BF16, tag="xtall")
                    ncopy = 0
                    for g in range(NT // 2):   # 2 t-tiles per group
                        tpw = ps_xt.tile([P, 2 * NC], F32R, tag="psxt")
                        for j in range(2):
                            t = 2 * g + j
                            for s in range(NSUB):
                                nc.tensor.transpose(
                                    tpw[:, j * NC + s * P : j * NC + (s + 1) * P],
                                    xr[s][:, t * P : (t + 1) * P],
                                    ident_fr[:],
                                )
                        # Pool/GpSimd cannot touch PSUM: DVE + ScalarE only
                        if ncopy % 2:
                            nc.scalar.activation(
                                xt_all[:, g * 2 * NC : (g + 1) * 2 * NC], tpw[:],
                                ACTF.Copy,
                            )
                        else:
                            nc.vector.tensor_copy(
                                xt_all[:, g * 2 * NC : (g + 1) * 2 * NC], tpw[:]
                            )
                        ncopy += 1

                    # mm1: encT[:, chunk] = tanh(W_enc^T x^T + b_enc)
                    for k in range(ND):
                        ps = ps_mm1.tile([P, NC], F32, tag="psmm1")
                        for t in range(NT):
                            nc.tensor.matmul(
                                ps[:], wenc[t][:, k * P : (k + 1) * P],
                                xt_all[:, t * NC : (t + 1) * NC],
                                start=(t == 0), stop=(t == NT - 1),
                            )
                        nc.scalar.activation(
                            encT[k][:, base : base + NC], ps[:], ACTF.Tanh,
                            bias=bencT[k][:],
                        )

                    # en natural (bf16) via PE transposes + nsq (pre-divided by D)
                    for s in range(NSUB):
                        i = c * NSUB + s
                        tpe = ps_en.tile([P, D], BF16, tag="psen")
                        for k in range(ND):
                            nc.tensor.transpose(
                                tpe[:, k * P : (k + 1) * P],
                                encT[k][:, base + s * P : base + (s + 1) * P],
                                ident_bf[:],
                            )
                        if s % 2:
                            nc.vector.tensor_copy(en_t[i][:], tpe[:])
                        else:
                            nc.scalar.activation(en_t[i][:], tpe[:], ACTF.Copy)
                        jn = junkp.tile([P, D], BF16, tag="junk")
                        nc.scalar.activation(
                            jn[:], en_t[i][:], ACTF.Square, scale=RSQD,
                            accum_out=nsq_strip[:, i : i + 1],
                        )
                    xrow_c = xrow_nxt if c + 1 < NCH else None

                # ---- segment sums + counts (chained PSUM accumulation) ----
                seg_ps = ps_seg.tile([C, D], F32)
                cnt_ps = ps_miscp.tile([C, 1], F32, tag="pscnt")
                for i in range(NN):
                    nc.tensor.matmul(
                        seg_ps[:], oh_bf[:, i * C : (i + 1) * C], en_t[i][:],
                        start=(i == 0), stop=(i == NN - 1),
                    )
                for i in range(NN):
                    nc.tensor.matmul(
                        cnt_ps[:], oh_bf[:, i * C : (i + 1) * C], ones_col_bf[:],
                        start=(i == 0), stop=(i == NN - 1),
                    )
                seg_sb = accp.tile([C, D], F32)
                nc.scalar.activation(seg_sb[:], seg_ps[:], ACTF.Copy)
                counts_sb = accp.tile([C, 1], F32)
                nc.scalar.activation(counts_sb[:], cnt_ps[:], ACTF.Copy)

                # AllReduce #1: [C, D+4] (seg sums + counts), kicked early
                bounce_in = dp.tile([C, D + 4], F32)
                bounce_out = dp.tile([C, D + 4], F32)
                zr3 = accp.tile([C, 3], F32)
                nc.any.memset(zr3[:], 0.0)
                nc.sync.dma_start(bounce_in[:, 0:D], seg_sb[:])
                nc.sync.dma_start(bounce_in[:, D : D + 1], counts_sb[:])
                nc.sync.dma_start(bounce_in[:, D + 1 : D + 4], zr3[:])
                nc.gpsimd.collective_compute(
                    "AllReduce",
                    ALU.add,
                    replica_groups=[list(range(NCORES))],
                    ins=[bounce_in[:].opt()],
                    outs=[bounce_out[:].opt()],
                )

                # ---- mm3 block: logits/softmax/CCE (overlaps AllReduce #1) ----
                expt_all = accp.tile([P, NN * C], F32)
                sume_all = accp.tile([P, NN], F32)
                for i in range(NN):
                    ps3 = ps_miscp.tile([P, C], F32, tag="psmm3")
                    for k in range(ND):
                        nc.tensor.matmul(
                            ps3[:], encT[k][:, i * P : (i + 1) * P], wcls[k][:],
                            start=(k == 0), stop=False,
                        )
                    nc.tensor.matmul(
                        ps3[:], ones_k1b[:], bcls_row[:], start=False, stop=True
                    )
                    nc.scalar.activation(
                        expt_all[:, i * C : (i + 1) * C], ps3[:], ACTF.Exp,
                        accum_out=sume_all[:, i : i + 1],
                    )
                rcp_all = accp.tile([P, NN], F32)
                nc.vector.reciprocal(rcp_all[:], sume_all[:])

                rs_all = accp.tile([P, NN], F32)
                nc.vector.tensor_reduce(
                    rs_all[:], catl_all[:].rearrange("p (i c) -> p i c", c=C),
                    AX.X, ALU.add,
                )
                rr_all = accp.tile([P, NN], F32)
                nc.vector.reciprocal(rr_all[:], rs_all[:])
                yp_all = accp.tile([P, NN * C], F32)
                for i in range(NN):
                    (nc.gpsimd if i % 2 else nc.vector).tensor_scalar(
                        out=yp_all[:, i * C : (i + 1) * C],
                        in0=catl_all[:, i * C : (i + 1) * C],
                        scalar1=rr_all[:, i : i + 1], scalar2=None, op0=ALU.mult,
                    )
                yc_all = accp.tile([P, NN * C], F32)
                nc.gpsimd.tensor_scalar(
                    out=yc_all[:], in0=yp_all[:],
                    scalar1=KEPS, scalar2=1.0 - KEPS, op0=ALU.max, op1=ALU.min,
                )
                lg_all = accp.tile([P, NN * C], F32)
                nc.scalar.activation(lg_all[:], yc_all[:], ACTF.Ln)
                pr_all = accp.tile([P, NN * C], F32)
                nc.vector.tensor_tensor(pr_all[:], expt_all[:], lg_all[:], ALU.mult)
                t1_all = accp.tile([P, NN], F32)
                nc.vector.tensor_reduce(
                    t1_all[:], pr_all[:].rearrange("p (i c) -> p i c", c=C),
                    AX.X, ALU.add,
                )
                # cce = -t1/sume ; minus sign folded into the final coefficient
                nc.vector.tensor_tensor(cat_strip[:], t1_all[:], rcp_all[:], ALU.mult)

            # ======== PASS B: mm2 -> dct(+bias), rec pinball, mm4 ========
            with tc.tile_pool(name="ps_mm2", bufs=2, space="PSUM") as ps_mm2, \
                 tc.tile_pool(name="ps_dt", bufs=2, space="PSUM") as ps_dt, \
                 tc.tile_pool(name="ps_mm4", bufs=2, space="PSUM") as ps_mm4, \
                 tc.tile_pool(name="dct", bufs=2) as dctp, \
                 tc.tile_pool(name="obf", bufs=2) as obfp, \
                 tc.tile_pool(name="dd", bufs=3) as ddp, \
                 tc.tile_pool(name="lt", bufs=2) as ltp:

                for c in range(NCH):
                    base = c * NC
                    orow = []
                    for s in range(NSUB):
                        r_ = orowp.tile([P, T], F32, tag=f"or{s}")
                        nc.sync.dma_start(
                            r_[:], o_d[base + s * P : base + (s + 1) * P, :]
                        )
                        rb = obfp.tile([P, T], BF16, tag=f"ob{s}")
                        nc.gpsimd.tensor_copy(rb[:], r_[:])
                        orow.append(rb)

                    # mm2: decodedT + b_dec -> dct bf16
                    dct_all = dctp.tile([P, NT * NC], BF16, tag="dctall")
                    ncopy = 0
                    for t in range(NT):
                        ps = ps_mm2.tile([P, NC], F32, tag="psmm2")
                        for k in range(ND):
                            nc.tensor.matmul(
                                ps[:], wdec[k][:, t * P : (t + 1) * P],
                                encT[k][:, base : base + NC],
                                start=(k == 0), stop=(k == ND - 1),
                            )
                        if ncopy % 2:
                            nc.scalar.activation(
                                dct_all[:, t * NC : (t + 1) * NC], ps[:],
                                ACTF.Identity, bias=bdecT[t][:],
                            )
                        else:
                            nc.vector.tensor_scalar(
                                out=dct_all[:, t * NC : (t + 1) * NC], in0=ps[:],
                                scalar1=bdecT[t][:], scalar2=None, op0=ALU.add,
                            )
                        ncopy += 1

                    # rec pinball: transpose 4 consecutive t-tiles of one
                    # sample block into a [128,512] psum group -> ONE wide
                    # subtract vs the contiguous output slice -> abs-reduce.
                    for s in range(NSUB):
                        for g in range(NT // 4):      # 4 groups per s
                            tpd = ps_dt.tile([P, 4 * P], BF16, tag="psdt")
                            for j in range(4):
                                t = 4 * g + j
                                nc.tensor.transpose(
                                    tpd[:, j * P : (j + 1) * P],
                                    dct_all[:, t * NC + s * P : t * NC + (s + 1) * P],
                                    ident_bf[:],
                                )
                            dd = ddp.tile([P, 4 * P], BF16, tag="dd")
                            nc.vector.tensor_tensor(
                                dd[:], tpd[:],
                                orow[s][:, 4 * g * P : 4 * (g + 1) * P],
                                ALU.subtract,
                            )
                            col = c * 8 + s * 4 + g
                            nc.vector.tensor_reduce(
                                rec_strip[:, col : col + 1], dd[:],
                                AX.X, ALU.add, apply_absolute_value=True,
                            )

                    # mm4: rec_latents = tanh(decoded @ W_enc + b_enc); lat pinball
                    for s in range(NSUB):
                        i = c * NSUB + s
                        ps4 = ps_mm4.tile([P, D], F32, tag="psmm4")
                        for t in range(NT):
                            nc.tensor.matmul(
                                ps4[:],
                                dct_all[:, t * NC + s * P : t * NC + (s + 1) * P],
                                wenc[t][:], start=(t == 0), stop=False,
                            )
                        nc.tensor.matmul(
                            ps4[:], ones_k1b[:], benc_row[:], start=False, stop=True
                        )
                        lt = ltp.tile([P, D], BF16, tag="lt")
                        nc.scalar.activation(lt[:], ps4[:], ACTF.Tanh)
                        d2 = ddp.tile([P, D], BF16, tag="dd2")
                        (nc.vector if s % 2 else nc.gpsimd).tensor_tensor(
                            d2[:], lt[:], en_t[i][:], ALU.subtract
                        )
                        nc.vector.tensor_reduce(
                            lat_strip[:, i : i + 1], d2[:], AX.X, ALU.add,
                            apply_absolute_value=True,
                        )

            # ======== means prep (AllReduce #1 done long ago) ========
            with tc.tile_pool(name="ps_p2", bufs=2, space="PSUM") as ps_p2, \
                 tc.tile_pool(name="ps_q", bufs=2, space="PSUM") as ps_q, \
                 tc.tile_pool(name="p2s", bufs=2) as p2s:

                sums_g = accp.tile([C, D], F32)
                nc.sync.dma_start(sums_g[:], bounce_out[:, 0:D])
                counts_g = accp.tile([C, 1], F32)
                nc.sync.dma_start(counts_g[:], bounce_out[:, D : D + 1])

                cmax = accp.tile([C, 1], F32)
                nc.vector.tensor_scalar(
                    out=cmax[:], in0=counts_g[:], scalar1=1.0, scalar2=None,
                    op0=ALU.max,
                )
                crcp = accp.tile([C, 1], F32)
                nc.vector.reciprocal(crcp[:], cmax[:])
                means = accp.tile([C, D], F32)
                nc.vector.tensor_scalar(
                    out=means[:], in0=sums_g[:], scalar1=crcp[:], scalar2=None,
                    op0=ALU.mult,
                )
                # msq pre-divided by D via scale inside Square
                msq_col = accp.tile([C, 1], F32)
                jm = junkp.tile([C, D], BF16, tag="junkm")
                nc.scalar.activation(
                    jm[:], means[:], ACTF.Square, scale=RSQD, accum_out=msq_col[:]
                )

                meansT = []
                for k in range(ND):
                    tpm = ps_p2.tile([P, C], F32, tag="psp2")
                    nc.tensor.transpose(
                        tpm[:], means[:, k * P : (k + 1) * P], ident_f32[:C, :C]
                    )
                    mt = p2s.tile([P, C], BF16, tag=f"mT{k}")
                    nc.vector.tensor_copy(mt[:], tpm[:])
                    meansT.append(mt)

                tpq = ps_p2.tile([1, C], F32, tag="psp2b")
                nc.tensor.transpose(tpq[:], msq_col[:], ident_f32[:C, :C])
                msq_row4 = p2s.tile([1, 4 * C], F32, tag="msqr")
                for j in range(4):
                    nc.vector.tensor_copy(msq_row4[:, j * C : (j + 1) * C], tpq[:])
                psb4 = ps_p2.tile([P, 4 * C], F32, tag="psp2")
                nc.tensor.matmul(
                    psb4[:], ones_k1f[:], msq_row4[:], start=True, stop=True
                )
                msq_b4 = p2s.tile([P, 4 * C], F32, tag="msqb")
                nc.scalar.activation(msq_b4[:], psb4[:], ACTF.Copy)

                # ---- scalar partials -> AllReduce #2 (tiny) ----
                pk = accp.tile([P, 3], F32)
                nc.vector.tensor_reduce(pk[:, 0:1], rec_strip[:], AX.X, ALU.add)
                nc.vector.tensor_reduce(pk[:, 1:2], lat_strip[:], AX.X, ALU.add)
                nc.vector.tensor_reduce(pk[:, 2:3], cat_strip[:], AX.X, ALU.add)
                scps = ps_p2.tile([1, 3], F32, tag="psp2b")
                nc.tensor.matmul(scps[:], ones_col[:], pk[:], start=True, stop=True)
                sc_row = accp.tile([1, 3], F32)
                nc.scalar.activation(sc_row[:], scps[:], ACTF.Copy)

                b2_in = dp.tile([1, 8], F32)
                b2_out = dp.tile([1, 8], F32)
                zr8 = accp.tile([1, 8], F32)
                nc.any.memset(zr8[:], 0.0)
                nc.sync.dma_start(b2_in[:], zr8[:])
                nc.sync.dma_start(b2_in[:, 0:3], sc_row[:])
                nc.gpsimd.collective_compute(
                    "AllReduce",
                    ALU.add,
                    replica_groups=[list(range(NCORES))],
                    ins=[b2_in[:].opt()],
                    outs=[b2_out[:].opt()],
                )

                # ---- phase 2 q-loop (overlaps AllReduce #2) ----
                # q = (-2*enc.mean_c + msq_c)/D gathered at the sample's class
                gq_strip = accp.tile([P, NN], F32)
                for g in range(NN // 4):
                    psq = ps_q.tile([P, 4 * C], F32, tag="psq")
                    for j in range(4):
                        i = 4 * g + j
                        for k in range(ND):
                            nc.tensor.matmul(
                                psq[:, j * C : (j + 1) * C],
                                encT[k][:, i * P : (i + 1) * P], meansT[k][:],
                                start=(k == 0), stop=(k == ND - 1),
                            )
                    qt = p2s.tile([P, 4 * C], F32, tag="qt")
                    nc.vector.scalar_tensor_tensor(
                        out=qt[:], in0=psq[:], scalar=-2.0 * RSQD * RSQD,
                        in1=msq_b4[:], op0=ALU.mult, op1=ALU.add,
                    )
                    j4 = p2s.tile([P, 4 * C], F32, tag="j4")
                    nc.gpsimd.tensor_tensor(
                        j4[:], qt[:], oh_all[:, g * 4 * C : (g + 1) * 4 * C], ALU.mult
                    )
                    nc.vector.tensor_reduce(
                        gq_strip[:, g * 4 : (g + 1) * 4],
                        j4[:].rearrange("p (i c) -> p i c", c=C),
                        AX.X, ALU.add,
                    )
                t2_strip = accp.tile([P, NN], F32)
                nc.vector.tensor_tensor(t2_strip[:], gq_strip[:], nsq_strip[:], ALU.add)

                # ---- final: + global scalar, transpose, single output DMA ----
                sc_g = accp.tile([1, 3], F32)
                nc.sync.dma_start(sc_g[:], b2_out[:, 0:3])
                coef = accp.tile([1, 3], F32)
                nc.any.memset(coef[:, 0:1], 0.9 / (float(n_global) * T))
                nc.any.memset(coef[:, 1:2], 0.9 / (float(n_global) * D))
                nc.any.memset(coef[:, 2:3], -1.0 / float(n_global))
                sprod = accp.tile([1, 3], F32)
                nc.vector.tensor_tensor(sprod[:], sc_g[:], coef[:], ALU.mult)
                stot = accp.tile([1, 1], F32)
                nc.vector.tensor_reduce(stot[:], sprod[:], AX.X, ALU.add)
                psS = ps_p2.tile([P, 1], F32, tag="psp2b")
                nc.tensor.matmul(psS[:], ones_k1f[:], stot[:], start=True, stop=True)
                s_col = accp.tile([P, 1], F32)
                nc.scalar.activation(s_col[:], psS[:], ACTF.Copy)

                out_strip = accp.tile([P, NN], F32)
                nc.vector.tensor_scalar(
                    out=out_strip[:], in0=t2_strip[:],
                    scalar1=s_col[:], scalar2=None, op0=ALU.add,
                )
                psT = ps_p2.tile([NN, P], F32, tag="psp2")
                nc.tensor.transpose(psT[:], out_strip[:], ident_f32[:])
                outT = accp.tile([NN, P], F32)
                nc.scalar.activation(outT[:], psT[:], ACTF.Copy)
                nc.sync.dma_start(
                    out_d.ap().rearrange("(a p) -> a p", p=P), outT[:]
                )

    nc.compile()
    return nc


_CACHE = {}


def _get_nc():
    if "nc" not in _CACHE:
        _CACHE["nc"] = build()
    return _CACHE["nc"]


def kernel(**inputs):
    nc = _get_nc()
    nl = N_GLOBAL // NCORES
    shard_names = ["x", "output", "cat_labels", "labels"]
    full_names = ["W_enc", "b_enc", "W_dec", "b_dec", "W_cls", "b_cls"]
    in_maps = []
    for i in range(NCORES):
        m = {}
        for k in shard_names:
            m[k] = np.ascontiguousarray(inputs[k][i * nl : (i + 1) * nl])
        for k in full_names:
            m[k] = np.ascontiguousarray(inputs[k])
        in_maps.append(m)
    res = run_bass_kernel_spmd(nc, in_maps, list(range(NCORES))).results
    return np.concatenate([res[i]["out"] for i in range(NCORES)]).astype(np.float32)


# revision 29
# speedup vs baseline: 1.5602x; 1.1102x over previous
"""Trainium2 Bass kernel for nn_Custom_trainer_79242146611896 (v3).

Data-parallel over N=16384 samples across 8 NeuronCores (2048/core).

Structure:
  - all matmuls/transposes in bf16 (1 cyc/row); x / output transposed in
    f32r mode (1.5 cyc/row, exact)
  - W_dec @ W_enc fused into WW [D,D] once at ramp: rec_latents =
    tanh(enc @ WW + (b_enc + b_dec @ W_enc)) -- kills the big mm4
  - rec diff built entirely inside PSUM: decodedT chain += b_dec (row
    matmul) -= outputT (neg-identity matmul); decoded never hits SBUF.
    One abs-reduce per [128,512] PSUM group.
  - pass A (mm1 -> encT/en + seg sums) kicks the [C,D+4] AllReduce early
    so it hides under the mm3/CCE block and pass B; a second tiny
    AllReduce for the 3 scalar partials hides under means-prep, the nsq
    block and the phase-2 q-loop
  - Ln ops grouped: 2 activation-table swaps total
  - GpSimd/Pool only does iota + collectives (its ALU is far too slow)
"""

import numpy as np

import concourse.bass as bass
import concourse.mybir as mybir
import concourse.tile as tile
from concourse import bacc
from concourse.bass_utils import run_bass_kernel_spmd
from concourse.masks import make_identity

F32 = mybir.dt.float32
F32R = mybir.dt.float32r
BF16 = mybir.dt.bfloat16
I32 = mybir.dt.int32
AX = mybir.AxisListType
ALU = mybir.AluOpType
ACTF = mybir.ActivationFunctionType

P = 128
NCORES = 8
N_GLOBAL = 16384
T = 2048
D = 512
C = 50
KEPS = 1e-7


def build(nl=N_GLOBAL // NCORES, nc_chunk=256, n_global=None):
    n_global = n_global or NCORES * nl
    NT = T // P          # 16 T-tiles
    ND = D // P          # 4 D-tiles
    NN = nl // P         # 16 sample-tiles per core
    NC = nc_chunk        # samples per chunk (256)
    NCH = nl // NC       # 8 chunks
    NSUB = NC // P       # 2 sample-tiles per chunk
    RSQD = 1.0 / float(np.sqrt(D))   # nsq/msq come out pre-divided by D

    nc = bacc.Bacc("TRN2", target_bir_lowering=False, debug=False, num_devices=NCORES)

    # x, output declared f32r so PE transposes run at 1.5 cyc/row (exact)
    x_d = nc.dram_tensor("x", [nl, T], F32R, kind="ExternalInput")
    o_d = nc.dram_tensor("output", [nl, T], F32R, kind="ExternalInput")
    cl_d = nc.dram_tensor("cat_labels", [nl, C], F32, kind="ExternalInput")
    lab_d = nc.dram_tensor("labels", [nl], I32, kind="ExternalInput")
    wenc_d = nc.dram_tensor("W_enc", [T, D], F32, kind="ExternalInput")
    benc_d = nc.dram_tensor("b_enc", [D], F32, kind="ExternalInput")
    wdec_d = nc.dram_tensor("W_dec", [D, T], F32, kind="ExternalInput")
    bdec_d = nc.dram_tensor("b_dec", [T], F32, kind="ExternalInput")
    wcls_d = nc.dram_tensor("W_cls", [D, C], F32, kind="ExternalInput")
    bcls_d = nc.dram_tensor("b_cls", [C], F32, kind="ExternalInput")
    out_d = nc.dram_tensor("out", [nl], F32, kind="ExternalOutput")

    from contextlib import ExitStack

    with tile.TileContext(nc) as tc:
        with ExitStack() as ctx:
            ent = ctx.enter_context
            constp = ent(tc.tile_pool(name="const", bufs=1))
            wts = ent(tc.tile_pool(name="wts", bufs=1))      # persistent weights
            encp = ent(tc.tile_pool(name="enc", bufs=1))     # encT + en (persistent)
            accp = ent(tc.tile_pool(name="acc", bufs=1))     # strips, wides, onehot
            junkp = ent(tc.tile_pool(name="junk", bufs=2))
            dp = ent(tc.tile_pool(name="dram", bufs=1, space="DRAM"))

            # ---------------- constants ----------------
            ident_f32 = constp.tile([P, P], F32)
            make_identity(nc, ident_f32)
            ident_bf = constp.tile([P, P], BF16)
            nc.vector.tensor_copy(ident_bf[:], ident_f32[:])
            ident_fr = constp.tile([P, P], F32R)
            nc.vector.tensor_copy(ident_fr[:], ident_f32[:])
            nident_bf = constp.tile([P, P], BF16)
            nc.vector.tensor_scalar(
                out=nident_bf[:], in0=ident_f32[:], scalar1=-1.0, scalar2=None,
                op0=ALU.mult,
            )

            ones_col = constp.tile([P, 1], F32)
            nc.any.memset(ones_col[:], 1.0)
            ones_col_bf = constp.tile([P, 1], BF16)
            nc.any.memset(ones_col_bf[:], 1.0)
            ones_k1f = constp.tile([1, P], F32)
            nc.any.memset(ones_k1f[:], 1.0)
            ones_k1b = constp.tile([1, P], BF16)
            nc.any.memset(ones_k1b[:], 1.0)
            ones_row2 = constp.tile([1, NC], BF16)
            nc.any.memset(ones_row2[:], 1.0)

            iot = constp.tile([P, C], I32)
            nc.gpsimd.iota(iot[:], [[1, C]], channel_multiplier=0)
            iotaf = constp.tile([P, C], F32)
            nc.vector.tensor_copy(iotaf[:], iot[:])

            # persistent big tensors
            encT = [encp.tile([P, nl], BF16, name=f"encT{k}", tag=f"encT{k}")
                    for k in range(ND)]
            en_t = [encp.tile([P, D], BF16, name=f"en{i}", tag=f"en{i}")
                    for i in range(NN)]

            nsq_strip = accp.tile([P, NN], F32)
            rec_strip = accp.tile([P, NCH * 8], F32)
            lat_strip = accp.tile([P, NN], F32)
            cat_strip = accp.tile([P, NN], F32)

            # ======== ramp + PASS A ========
            with tc.tile_pool(name="stg", bufs=1) as stg, \
                 tc.tile_pool(name="wdt", bufs=1) as wdtp, \
                 tc.tile_pool(name="xr", bufs=2) as xrp, \
                 tc.tile_pool(name="xt", bufs=2) as xtp, \
                 tc.tile_pool(name="ps_xt", bufs=2, space="PSUM") as ps_xt, \
                 tc.tile_pool(name="ps_mm1", bufs=2, space="PSUM") as ps_mm1, \
                 tc.tile_pool(name="ps_en", bufs=1, space="PSUM") as ps_en, \
                 tc.tile_pool(name="ps_seg", bufs=1, space="PSUM") as ps_seg, \
                 tc.tile_pool(name="ps_misc", bufs=2, space="PSUM") as ps_miscp:

                def load_x(c):
                    rs = []
                    for s in range(NSUB):
                        r_ = xrp.tile([P, T], F32R, tag=f"xr{s}")
                        nc.sync.dma_start(
                            r_[:], x_d[c * NC + s * P : c * NC + (s + 1) * P, :]
                        )
                        rs.append(r_)
                    return rs

                xrow_c = load_x(0)

                # ---- W_enc first (mm1 needs it) ----
                wenc_r = wenc_d.ap().rearrange("(a p) d -> a p d", p=P)
                wenc = []
                for t in range(NT):
                    s_ = stg.tile([P, D], F32, tag="stgd")
                    nc.sync.dma_start(s_[:], wenc_r[t])
                    w_ = wts.tile([P, D], BF16, tag=f"wenc{t}")
                    if t % 2:
                        nc.scalar.activation(w_[:], s_[:], ACTF.Copy)
                    else:
                        nc.vector.tensor_copy(w_[:], s_[:])
                    wenc.append(w_)

                benc_r = benc_d.ap().rearrange("(a p) -> a p", p=P)
                bencT = []
                for k in range(ND):
                    b_ = wts.tile([P, 1], F32, tag=f"bencT{k}")
                    nc.sync.dma_start(b_[:], benc_r[k].rearrange("(p o) -> p o", o=1))
                    bencT.append(b_)

                wdec_r = wdec_d.ap().rearrange("(a p) t -> a p t", p=P)
                wdec = []
                for k in range(ND):
                    s_ = stg.tile([P, T], F32, tag="stgt")
                    nc.sync.dma_start(s_[:], wdec_r[k])
                    w_ = wts.tile([P, T], BF16, tag=f"wdec{k}")
                    if k % 2:
                        nc.scalar.activation(w_[:], s_[:], ACTF.Copy)
                    else:
                        nc.vector.tensor_copy(w_[:], s_[:])
                    wdec.append(w_)

                # ---- labels -> onehot; catl (needed later, overlaps) ----
                labi_all = accp.tile([P, NN], I32)
                for i in range(NN):
                    nc.sync.dma_start(
                        labi_all[:, i : i + 1],
                        lab_d[i * P : (i + 1) * P].rearrange("(p o) -> p o", o=1),
                    )
                labf_all = accp.tile([P, NN], F32)
                nc.vector.tensor_copy(labf_all[:], labi_all[:])
                oh_all = accp.tile([P, NN * C], F32)
                oh_bf = accp.tile([P, NN * C], BF16)
                for i in range(NN):
                    nc.vector.tensor_scalar(
                        out=oh_all[:, i * C : (i + 1) * C], in0=iotaf[:],
                        scalar1=labf_all[:, i : i + 1], scalar2=None, op0=ALU.is_equal,
                    )
                nc.vector.tensor_copy(oh_bf[:], oh_all[:])
                catl_all = accp.tile([P, NN * C], F32)
                nc.sync.dma_start(
                    catl_all[:], cl_d.ap().rearrange("(a p) c -> p a c", p=P)
                )

                benc_row_f = stg.tile([1, D], F32, tag="stgbr")
                nc.sync.dma_start(
                    benc_row_f[:], benc_d.ap().rearrange("(o d) -> o d", o=1)
                )
                bdec_row_f = stg.tile([1, T], F32, tag="stgdr")
                nc.sync.dma_start(
                    bdec_row_f[:], bdec_d.ap().rearrange("(o t) -> o t", o=1)
                )
                bdec_row = wts.tile([1, T], BF16)
                nc.vector.tensor_copy(bdec_row[:], bdec_row_f[:])
                bdec_r = bdec_d.ap().rearrange("(a p) -> a p", p=P)
                bdecT = []
                for t in range(NT):
                    s_ = stg.tile([P, 1], F32, tag="stgb")
                    nc.sync.dma_start(s_[:], bdec_r[t].rearrange("(p o) -> p o", o=1))
                    b_ = wts.tile([P, 1], BF16, tag=f"bdecT{t}")
                    nc.vector.tensor_copy(b_[:], s_[:])
                    bdecT.append(b_)

                wcls_r = wcls_d.ap().rearrange("(a p) c -> a p c", p=P)
                wcls = []
                for k in range(ND):
                    s_ = stg.tile([P, C], F32, tag="stgc")
                    nc.sync.dma_start(s_[:], wcls_r[k])
                    w_ = wts.tile([P, C], BF16, tag=f"wcls{k}")
                    nc.vector.tensor_copy(w_[:], s_[:])
                    wcls.append(w_)
                bcls_row_f = stg.tile([1, C], F32, tag="stgcr")
                nc.sync.dma_start(
                    bcls_row_f[:], bcls_d.ap().rearrange("(o c) -> o c", o=1)
                )
                bcls_row = wts.tile([1, C], BF16)
                nc.vector.tensor_copy(bcls_row[:], bcls_row_f[:])

                # ---- WW = W_dec @ W_enc (bf16), cr = b_enc + b_dec @ W_enc ----
                wdecT = []
                for t in range(NT):
                    tpw = ps_en.tile([P, D], BF16, tag="psen")
                    for k in range(ND):
                        nc.tensor.transpose(
                            tpw[:, k * P : (k + 1) * P],
                            wdec[k][:, t * P : (t + 1) * P], ident_bf[:],
                        )
                    w_ = wdtp.tile([P, D], BF16, tag=f"wdecT{t}")
                    if t % 2:
                        nc.scalar.activation(w_[:], tpw[:], ACTF.Copy)
                    else:
                        nc.vector.tensor_copy(w_[:], tpw[:])
                    wdecT.append(w_)
                WW = []
                for g1 in range(ND):
                    psw = ps_mm1.tile([P, D], F32, tag="psmm1")
                    for t in range(NT):
                        nc.tensor.matmul(
                            psw[:], wdecT[t][:, g1 * P : (g1 + 1) * P], wenc[t][:],
                            start=(t == 0), stop=(t == NT - 1),
                        )
                    w_ = wts.tile([P, D], BF16, tag=f"WW{g1}")
                    if g1 % 2:
                        nc.scalar.activation(w_[:], psw[:], ACTF.Copy)
                    else:
                        nc.vector.tensor_copy(w_[:], psw[:])
                    WW.append(w_)
                pscr_t = ps_mm1.tile([P, D], F32, tag="psmm1")
                pscr = pscr_t[0:1, :]
                for t in range(NT):
                    nc.tensor.matmul(
                        pscr, bdecT[t][:], wenc[t][:],
                        start=(t == 0), stop=(t == NT - 1),
                    )
                crf = stg.tile([1, D], F32, tag="stgbr2")
                nc.vector.tensor_tensor(crf[:], pscr, benc_row_f[:], ALU.add)
                cr_row = wts.tile([1, D], BF16)
                nc.vector.tensor_copy(cr_row[:], crf[:])

                # ---- PASS A chunks: x^T, mm1 -> encT, en ----
                for c in range(NCH):
                    base = c * NC
                    xr = xrow_c
                    if c + 1 < NCH:
                        xrow_nxt = load_x(c + 1)

                    xt_all = xtp.tile([P, NT * NC], BF16, tag="xtall")
                    ncopy = 0
                    for g in range(NT // 2):
                        tpw = ps_xt.tile([P, 2 * NC], F32R, tag="psxt")
                        for j in range(2):
                            t = 2 * g + j
                            for s in range(NSUB):
                                nc.tensor.transpose(
                                    tpw[:, j * NC + s * P : j * NC + (s + 1) * P],
                                    xr[s][:, t * P : (t + 1) * P],
                                    ident_fr[:],
                                )
                        if ncopy % 2:
                            nc.scalar.activation(
                                xt_all[:, g * 2 * NC : (g + 1) * 2 * NC], tpw[:],
                                ACTF.Copy,
                            )
                        else:
                            nc.vector.tensor_copy(
                                xt_all[:, g * 2 * NC : (g + 1) * 2 * NC], tpw[:]
                            )
                        ncopy += 1

                    for k in range(ND):
                        ps_t = ps_mm1.tile([P, D], F32, tag="psmm1")
                        ps = ps_t[:, 0:NC]
                        for t in range(NT):
                            nc.tensor.matmul(
                                ps, wenc[t][:, k * P : (k + 1) * P],
                                xt_all[:, t * NC : (t + 1) * NC],
                                start=(t == 0), stop=(t == NT - 1),
                            )
                        nc.scalar.activation(
                            encT[k][:, base : base + NC], ps, ACTF.Tanh,
                            bias=bencT[k][:],
                        )

                    for s in range(NSUB):
                        i = c * NSUB + s
                        tpe = ps_en.tile([P, D], BF16, tag="psen")
                        for k in range(ND):
                            nc.tensor.transpose(
                                tpe[:, k * P : (k + 1) * P],
                                encT[k][:, base + s * P : base + (s + 1) * P],
                                ident_bf[:],
                            )
                        if s % 2:
                            nc.vector.tensor_copy(en_t[i][:], tpe[:])
                        else:
                            nc.scalar.activation(en_t[i][:], tpe[:], ACTF.Copy)
                    xrow_c = xrow_nxt if c + 1 < NCH else None

                # ---- segment sums + counts (chained PSUM accumulation) ----
                seg_ps = ps_seg.tile([C, D], F32)
                cnt_t = ps_miscp.tile([P, D], F32, tag="psmisc")
                cnt_ps = cnt_t[0:C, 0:1]
                for i in range(NN):
                    nc.tensor.matmul(
                        seg_ps[:], oh_bf[:, i * C : (i + 1) * C], en_t[i][:],
                        start=(i == 0), stop=(i == NN - 1),
                    )
                for i in range(NN):
                    nc.tensor.matmul(
                        cnt_ps, oh_bf[:, i * C : (i + 1) * C], ones_col_bf[:],
                        start=(i == 0), stop=(i == NN - 1),
                    )
                seg_sb = accp.tile([C, D], F32)
                nc.scalar.activation(seg_sb[:], seg_ps[:], ACTF.Copy)
                counts_sb = accp.tile([C, 1], F32)
                nc.scalar.activation(counts_sb[:], cnt_ps, ACTF.Copy)

                # AllReduce #1: [C, D+4] (seg sums + counts), kicked early
                bounce_in = dp.tile([C, D + 4], F32)
                bounce_out = dp.tile([C, D + 4], F32)
                zr3 = accp.tile([C, 3], F32)
                nc.any.memset(zr3[:], 0.0)
                nc.sync.dma_start(bounce_in[:, 0:D], seg_sb[:])
                nc.sync.dma_start(bounce_in[:, D : D + 1], counts_sb[:])
                nc.sync.dma_start(bounce_in[:, D + 1 : D + 4], zr3[:])
                nc.gpsimd.collective_compute(
                    "AllReduce",
                    ALU.add,
                    replica_groups=[list(range(NCORES))],
                    ins=[bounce_in[:].opt()],
                    outs=[bounce_out[:].opt()],
                )

                # ---- mm3 block: logits/softmax/CCE (overlaps AllReduce #1) ----
                expt_all = accp.tile([P, NN * C], F32)
                sume_all = accp.tile([P, NN], F32)
                for i in range(NN):
                    ps3_t = ps_miscp.tile([P, D], F32, tag="psmisc")
                    ps3 = ps3_t[:, 0:C]
                    for k in range(ND):
                        nc.tensor.matmul(
                            ps3, encT[k][:, i * P : (i + 1) * P], wcls[k][:],
                            start=(k == 0), stop=False,
                        )
                    nc.tensor.matmul(
                        ps3, ones_k1b[:], bcls_row[:], start=False, stop=True
                    )
                    nc.scalar.activation(
                        expt_all[:, i * C : (i + 1) * C], ps3, ACTF.Exp,
                        accum_out=sume_all[:, i : i + 1],
                    )
                rcp_all = accp.tile([P, NN], F32)
                nc.vector.reciprocal(rcp_all[:], sume_all[:])

                rs_all = accp.tile([P, NN], F32)
                nc.vector.tensor_reduce(
                    rs_all[:], catl_all[:].rearrange("p (i c) -> p i c", c=C),
                    AX.X, ALU.add,
                )
                rr_all = accp.tile([P, NN], F32)
                nc.vector.reciprocal(rr_all[:], rs_all[:])
                yp_all = accp.tile([P, NN * C], F32)
                for i in range(NN):
                    nc.vector.tensor_scalar(
                        out=yp_all[:, i * C : (i + 1) * C],
                        in0=catl_all[:, i * C : (i + 1) * C],
                        scalar1=rr_all[:, i : i + 1], scalar2=None, op0=ALU.mult,
                    )
                yc_all = accp.tile([P, NN * C], F32)
                nc.vector.tensor_scalar(
                    out=yc_all[:], in0=yp_all[:],
                    scalar1=KEPS, scalar2=1.0 - KEPS, op0=ALU.max, op1=ALU.min,
                )
                lg_all = accp.tile([P, NN * C], F32)
                nc.scalar.activation(lg_all[:], yc_all[:], ACTF.Ln)
                pr_all = accp.tile([P, NN * C], F32)
                nc.vector.tensor_tensor(pr_all[:], expt_all[:], lg_all[:], ALU.mult)
                t1_all = accp.tile([P, NN], F32)
                nc.vector.tensor_reduce(
                    t1_all[:], pr_all[:].rearrange("p (i c) -> p i c", c=C),
                    AX.X, ALU.add,
                )
                nc.vector.tensor_tensor(cat_strip[:], t1_all[:], rcp_all[:], ALU.mult)

            # ======== PASS B: rec diff in PSUM + fused rec_latents ========
            with tc.tile_pool(name="ps_ot", bufs=2, space="PSUM") as ps_ot, \
                 tc.tile_pool(name="ps_dd", bufs=2, space="PSUM") as ps_dd, \
                 tc.tile_pool(name="ps_m4", bufs=2, space="PSUM") as ps_m4, \
                 tc.tile_pool(name="orow", bufs=2) as orowp, \
                 tc.tile_pool(name="ots", bufs=3) as otsp, \
                 tc.tile_pool(name="lt", bufs=2) as ltp, \
                 tc.tile_pool(name="d2p", bufs=2) as d2p:

                for c in range(NCH):
                    base = c * NC
                    orow = []
                    for s in range(NSUB):
                        r_ = orowp.tile([P, T], F32R, tag=f"or{s}")
                        nc.sync.dma_start(
                            r_[:], o_d[base + s * P : base + (s + 1) * P, :]
                        )
                        orow.append(r_)

                    for g in range(NT // 2):   # 2 t-tiles per group
                        # outputT for t=2g, 2g+1 -> [128, 512] psum -> sbuf bf16
                        tpo = ps_ot.tile([P, 2 * NC], F32R, tag="psot")
                        for j in range(2):
                            t = 2 * g + j
                            for s in range(NSUB):
                                nc.tensor.transpose(
                                    tpo[:, j * NC + s * P : j * NC + (s + 1) * P],
                                    orow[s][:, t * P : (t + 1) * P],
                                    ident_fr[:],
                                )
                        ot = otsp.tile([P, 2 * NC], BF16, tag="ot")
                        if g % 2:
                            nc.scalar.activation(ot[:], tpo[:], ACTF.Copy)
                        else:
                            nc.vector.tensor_copy(ot[:], tpo[:])

                        # decodedT + b_dec - outputT, entirely in PSUM
                        dd = ps_dd.tile([P, 2 * NC], F32, tag="psdd")
                        for j in range(2):
                            t = 2 * g + j
                            sl = slice(j * NC, (j + 1) * NC)
                            for k in range(ND):
                                nc.tensor.matmul(
                                    dd[:, sl], wdec[k][:, t * P : (t + 1) * P],
                                    encT[k][:, base : base + NC],
                                    start=(k == 0), stop=False,
                                )
                            nc.tensor.matmul(
                                dd[:, sl], bdec_row[:, t * P : (t + 1) * P],
                                ones_row2[:], start=False, stop=False,
                            )
                            nc.tensor.matmul(
                                dd[:, sl], nident_bf[:], ot[:, sl],
                                start=False, stop=True,
                            )
                        col = c * 8 + g
                        if g % 2:
                            jb = junkp.tile([P, 2 * NC], BF16, tag="junkb")
                            nc.scalar.activation(
                                jb[:], dd[:], ACTF.Abs,
                                accum_out=rec_strip[:, col : col + 1],
                            )
                        else:
                            nc.vector.tensor_reduce(
                                rec_strip[:, col : col + 1], dd[:],
                                AX.X, ALU.add, apply_absolute_value=True,
                            )

                    # fused rec_latents = tanh(enc @ WW + cr); lat pinball
                    for s in range(NSUB):
                        i = c * NSUB + s
                        ps4 = ps_m4.tile([P, D], F32, tag="psm4")
                        for k in range(ND):
                            nc.tensor.matmul(
                                ps4[:],
                                encT[k][:, base + s * P : base + (s + 1) * P],
                                WW[k][:], start=(k == 0), stop=False,
                            )
                        nc.tensor.matmul(
                            ps4[:], ones_k1b[:], cr_row[:], start=False, stop=True
                        )
                        lt = ltp.tile([P, D], BF16, tag="lt")
                        nc.scalar.activation(lt[:], ps4[:], ACTF.Tanh)
                        d2 = d2p.tile([P, D], BF16, tag="d2")
                        nc.vector.tensor_tensor(d2[:], lt[:], en_t[i][:], ALU.subtract)
                        nc.vector.tensor_reduce(
                            lat_strip[:, i : i + 1], d2[:], AX.X, ALU.add,
                            apply_absolute_value=True,
                        )

            # ======== tail: CC#2 kicked first, then overlapping work ========
            with tc.tile_pool(name="ps_p2", bufs=2, space="PSUM") as ps_p2, \
                 tc.tile_pool(name="ps_q", bufs=2, space="PSUM") as ps_q, \
                 tc.tile_pool(name="p2s", bufs=2) as p2s:

                # scalar partials -> AllReduce #2 (tiny)
                pk = accp.tile([P, 3], F32)
                nc.vector.tensor_reduce(pk[:, 0:1], rec_strip[:], AX.X, ALU.add)
                nc.vector.tensor_reduce(pk[:, 1:2], lat_strip[:], AX.X, ALU.add)
                nc.vector.tensor_reduce(pk[:, 2:3], cat_strip[:], AX.X, ALU.add)
                scps = ps_p2.tile([1, 3], F32, tag="psp2b")
                nc.tensor.matmul(scps[:], ones_col[:], pk[:], start=True, stop=True)
                sc_row = accp.tile([1, 3], F32)
                nc.scalar.activation(sc_row[:], scps[:], ACTF.Copy)

                b2_in = dp.tile([1, 8], F32)
                b2_out = dp.tile([1, 8], F32)
                zr8 = accp.tile([1, 8], F32)
                nc.any.memset(zr8[:], 0.0)
                nc.sync.dma_start(b2_in[:], zr8[:])
                nc.sync.dma_start(b2_in[:, 0:3], sc_row[:])
                nc.gpsimd.collective_compute(
                    "AllReduce",
                    ALU.add,
                    replica_groups=[list(range(NCORES))],
                    ins=[b2_in[:].opt()],
                    outs=[b2_out[:].opt()],
                )

                # means prep (AllReduce #1 result)
                sums_g = accp.tile([C, D], F32)
                nc.sync.dma_start(sums_g[:], bounce_out[:, 0:D])
                counts_g = accp.tile([C, 1], F32)
                nc.sync.dma_start(counts_g[:], bounce_out[:, D : D + 1])

                cmax = accp.tile([C, 1], F32)
                nc.vector.tensor_scalar(
                    out=cmax[:], in0=counts_g[:], scalar1=1.0, scalar2=None,
                    op0=ALU.max,
                )
                crcp = accp.tile([C, 1], F32)
                nc.vector.reciprocal(crcp[:], cmax[:])
                means = accp.tile([C, D], F32)
                nc.vector.tensor_scalar(
                    out=means[:], in0=sums_g[:], scalar1=crcp[:], scalar2=None,
                    op0=ALU.mult,
                )
                msq_col = accp.tile([C, 1], F32)
                jm = junkp.tile([C, D], BF16, tag="junkm")
                nc.scalar.activation(
                    jm[:], means[:], ACTF.Square, scale=RSQD, accum_out=msq_col[:]
                )

                meansT = []
                for k in range(ND):
                    tpm = ps_p2.tile([P, C], F32, tag="psp2")
                    nc.tensor.transpose(
                        tpm[:], means[:, k * P : (k + 1) * P], ident_f32[:C, :C]
                    )
                    mt = p2s.tile([P, C], BF16, tag=f"mT{k}")
                    nc.vector.tensor_copy(mt[:], tpm[:])
                    meansT.append(mt)

                tpq = ps_p2.tile([1, C], F32, tag="psp2b")
                nc.tensor.transpose(tpq[:], msq_col[:], ident_f32[:C, :C])
                msq_row4 = p2s.tile([1, 4 * C], F32, tag="msqr")
                for j in range(4):
                    nc.vector.tensor_copy(msq_row4[:, j * C : (j + 1) * C], tpq[:])
                psb4 = ps_p2.tile([P, 4 * C], F32, tag="psp2")
                nc.tensor.matmul(
                    psb4[:], ones_k1f[:], msq_row4[:], start=True, stop=True
                )
                msq_b4 = p2s.tile([P, 4 * C], F32, tag="msqb")
                nc.scalar.activation(msq_b4[:], psb4[:], ACTF.Copy)

                # nsq block (deferred here to overlap AllReduce #2)
                for i in range(NN):
                    jn = junkp.tile([P, D], BF16, tag="junk")
                    nc.scalar.activation(
                        jn[:], en_t[i][:], ACTF.Square, scale=RSQD,
                        accum_out=nsq_strip[:, i : i + 1],
                    )

                # phase 2 q-loop (also overlaps AllReduce #2)
                gq_strip = accp.tile([P, NN], F32)
                for g in range(NN // 4):
                    psq = ps_q.tile([P, 4 * C], F32, tag="psq")
                    for j in range(4):
                        i = 4 * g + j
                        for k in range(ND):
                            nc.tensor.matmul(
                                psq[:, j * C : (j + 1) * C],
                                encT[k][:, i * P : (i + 1) * P], meansT[k][:],
                                start=(k == 0), stop=(k == ND - 1),
                            )
                    qt = p2s.tile([P, 4 * C], F32, tag="qt")
                    nc.vector.scalar_tensor_tensor(
                        out=qt[:], in0=psq[:], scalar=-2.0 * RSQD * RSQD,
                        in1=msq_b4[:], op0=ALU.mult, op1=ALU.add,
                    )
                    j4 = p2s.tile([P, 4 * C], F32, tag="j4")
                    nc.vector.tensor_tensor(
                        j4[:], qt[:], oh_all[:, g * 4 * C : (g + 1) * 4 * C], ALU.mult
                    )
                    nc.vector.tensor_reduce(
                        gq_strip[:, g * 4 : (g + 1) * 4],
                        j4[:].rearrange("p (i c) -> p i c", c=C),
                        AX.X, ALU.add,
                    )
                t2_strip = accp.tile([P, NN], F32)
                nc.vector.tensor_tensor(t2_strip[:], gq_strip[:], nsq_strip[:], ALU.add)

                # final: + global scalar, transpose, single output DMA
                sc_g = accp.tile([1, 3], F32)
                nc.sync.dma_start(sc_g[:], b2_out[:, 0:3])
                coef = accp.tile([1, 3], F32)
                nc.any.memset(coef[:, 0:1], 0.9 / (float(n_global) * T))
                nc.any.memset(coef[:, 1:2], 0.9 / (float(n_global) * D))
                nc.any.memset(coef[:, 2:3], -1.0 / float(n_global))
                sprod = accp.tile([1, 3], F32)
                nc.vector.tensor_tensor(sprod[:], sc_g[:], coef[:], ALU.mult)
                stot = accp.tile([1, 1], F32)
                nc.vector.tensor_reduce(stot[:], sprod[:], AX.X, ALU.add)
                psS = ps_p2.tile([P, 1], F32, tag="psp2b")
                nc.tensor.matmul(psS[:], ones_k1f[:], stot[:], start=True, stop=True)
                s_col = accp.tile([P, 1], F32)
                nc.scalar.activation(s_col[:], psS[:], ACTF.Copy)

                out_strip = accp.tile([P, NN], F32)
                nc.vector.tensor_scalar(
                    out=out_strip[:], in0=t2_strip[:],
                    scalar1=s_col[:], scalar2=None, op0=ALU.add,
                )
                psT = ps_p2.tile([NN, P], F32, tag="psp2")
                nc.tensor.transpose(psT[:], out_strip[:], ident_f32[:])
                outT = accp.tile([NN, P], F32)
                nc.scalar.activation(outT[:], psT[:], ACTF.Copy)
                nc.sync.dma_start(
                    out_d.ap().rearrange("(a p) -> a p", p=P), outT[:]
                )

    nc.compile()
    return nc


_CACHE = {}


def _get_nc():
    if "nc" not in _CACHE:
        _CACHE["nc"] = build()
    return _CACHE["nc"]


def kernel(**inputs):
    nc = _get_nc()
    nl = N_GLOBAL // NCORES
    shard_names = ["x", "output", "cat_labels", "labels"]
    full_names = ["W_enc", "b_enc", "W_dec", "b_dec", "W_cls", "b_cls"]
    in_maps = []
    for i in range(NCORES):
        m = {}
        for k in shard_names:
            m[k] = np.ascontiguousarray(inputs[k][i * nl : (i + 1) * nl])
        for k in full_names:
            m[k] = np.ascontiguousarray(inputs[k])
        in_maps.append(m)
    res = run_bass_kernel_spmd(nc, in_maps, list(range(NCORES))).results
    return np.concatenate([res[i]["out"] for i in range(NCORES)]).astype(np.float32)
